# revision 22
# baseline (speedup 1.0000x reference)
"""CoLightNet Trainium2 Bass kernel (self-contained).

SPMD over 8 cores; core c owns output rows [c*1024, (c+1)*1024).
  inputs : stT   [S,N]    bf16  state transposed (host prep)
           stTm  [S,Mc]   bf16  own-rows slice of stT
           adjt  [N,Mc]   bf16  adjt[n,m] = adj[row m, col n]  (transposed)
           w1,w2,wqk,wh1 [128,128] bf16, wh2 [128,8] bf16
           b1,b2,bh1 [E,1] f32, bh2 [A,1] f32
  output : outb  [Mc,A]   f32

Math (identical to the reference, reformulated):
  hT    = w2^T relu(w1^T stT + b1) + b2                  # [E, N]
  qTp   = Wqk^T hTm            (Wqk = wq wk^T / sqrt(E)) # [E, Mc]
  sT    = hT-block (stationary) x qTp (moving)           # scores^T [N, Mc]
  e     = exp(sT);  w = e * adjt                         # post-exp mask
  den   = sum_n w   (PE matmul / DVE / GpSimd accumulators, merged in PSUM)
  aggT  = sum_blocks h-block (stationary) x w-block      # [E, Mc]
  out   = relu((aggT/den)^T wh1 + bh1) wh2 + bh2
"""

from contextlib import ExitStack

import concourse.bass as bass
import concourse.mybir as mybir
import concourse.tile as tile
from concourse import bacc
from concourse.masks import make_identity

F32 = mybir.dt.float32
F32R = mybir.dt.float32r
BF16 = mybir.dt.bfloat16
AF = mybir.ActivationFunctionType
ALU = mybir.AluOpType

S = 128
E = 128
A = 8


def ts(i, size):
    return slice(i * size, (i + 1) * size)


def ts_blk(j):
    return slice(j * 4, (j + 1) * 4)


# den ownership per n-block, period 8: PE x2, GpSimd x3, DVE x3
DEN_PAT = ["P", "G", "D", "G", "D", "G", "P", "D"]


def build_kernel(n_total=8192, m_core=1024):
    nc = bacc.Bacc("TRN2", debug=False)
    stT = nc.dram_tensor("stT", (S, n_total), BF16, kind="ExternalInput").ap()
    stTm = nc.dram_tensor("stTm", (S, m_core), BF16, kind="ExternalInput").ap()
    adjt = nc.dram_tensor("adjt", (n_total, m_core), BF16, kind="ExternalInput").ap()
    wt = {}
    for name, shape, dt in [
        ("w1", (S, E), BF16), ("w2", (E, E), BF16), ("wqk", (E, E), BF16),
        ("wh1", (E, E), BF16), ("wh2", (E, A), BF16),
        ("b1", (E, 1), F32), ("b2", (E, 1), F32),
        ("bh1", (E, 1), F32), ("bh2", (A, 1), F32),
    ]:
        wt[name] = nc.dram_tensor(name, shape, dt, kind="ExternalInput").ap()
    outb = nc.dram_tensor("outb", (m_core, A), F32, kind="ExternalOutput").ap()

    with tile.TileContext(nc) as tc:
        colight_body(tc, outb, stT, stTm, adjt, wt)
    nc.compile()
    return nc


def colight_body(tc, outb, stT, stTm, adjt, wt):
    nc = tc.nc
    n_total = stT.shape[1]
    m_core = adjt.shape[1]
    NCH = n_total // 512            # 512-wide n chunks (phase 1)
    NB = n_total // 128             # 128-wide n blocks (phase 2)
    MH = m_core // 512              # m halves

    with ExitStack() as ctx:
        singles = ctx.enter_context(tc.tile_pool(name="singles", bufs=1))

        # ---- staged inputs: big DMAs issued first (cheap issue, deep prefetch) ----
        stTm_sb = singles.tile([128, m_core], BF16, tag="stTm")
        nc.sync.dma_start(out=stTm_sb, in_=stTm)
        stT_sb = singles.tile([128, n_total], BF16, tag="stT")
        for q in range(4):
            nc.sync.dma_start(
                out=stT_sb[:, ts(q, n_total // 4)], in_=stT[:, ts(q, n_total // 4)]
            )

        # ---- constant weights ----
        wf = {}
        for name, shape, dt in [
            ("w1", [S, E], BF16), ("w2", [E, E], BF16), ("wqk", [E, E], BF16),
            ("wh1", [E, E], BF16), ("wh2", [E, A], BF16),
            ("b1", [E, 1], F32), ("b2", [E, 1], F32),
            ("bh1", [E, 1], F32), ("bh2", [A, 1], F32),
        ]:
            t = singles.tile(shape, dt, tag=f"w_{name}")
            nc.scalar.dma_start(out=t, in_=wt[name])
            wf[name] = t
        ones_col_f = singles.tile([128, 1], F32)    # vsum-reduce stationary
        nc.vector.memset(ones_col_f, 1.0)
        ones_col_r = singles.tile([128, 1], F32R)
        nc.vector.tensor_copy(out=ones_col_r, in_=ones_col_f)
        ones_row_f = singles.tile([1, 128], F32)    # outer-product stationary
        nc.vector.memset(ones_row_f, 1.0)
        ones_row_r = singles.tile([1, 128], F32R)
        nc.vector.tensor_copy(out=ones_row_r, in_=ones_row_f)
        ident_f = singles.tile([128, 128], F32)
        make_identity(nc, ident_f)
        ident_bf = singles.tile([128, 128], BF16)
        nc.vector.tensor_copy(out=ident_bf, in_=ident_f)

        # ---- persistent activations ----
        hT = singles.tile([128, NB, 128], BF16)     # h^T, [E, n]
        hblk = singles.tile([128, NB, 128], BF16)   # h normal, agg stationary
        qTp = singles.tile([128, MH, 512], BF16)    # Wqk^T hTm
        vsum_g = singles.tile([128, MH, 512], F32R)  # den accumulator (GpSimd folds)

        # ================= phase 1: MLP over all N =================
        ph1 = ExitStack()
        p1_sb = ph1.enter_context(tc.tile_pool(name="p1_sb", bufs=3))
        p1_ps = ph1.enter_context(tc.tile_pool(name="p1_ps", bufs=2, space="PSUM"))
        p1_tp = ph1.enter_context(tc.tile_pool(name="p1_tp", bufs=2, space="PSUM"))

        # one pipelined pass over MH own-row chunks (-> qTp) + NCH full chunks
        # (-> hT, hblk).  stage skew keeps every engine queue unblocked:
        #   step i: w1(i) | relu(i-1) | w2(i-1) | exit(i-1) | transp(i-2)
        NTOT = MH + NCH
        ps1s = {}
        h1s = {}
        ps2s = {}

        def src_dma(i):
            if i < MH:
                return stTm_sb[:, ts(i, 512)]
            return stT_sb[:, ts(i - MH, 512)]

        def p1_w1(i):
            st_in = src_dma(i)
            ps1 = p1_ps.tile([128, 512], F32, tag="pa")
            nc.tensor.matmul(ps1, wf["w1"], st_in, start=True, stop=True)
            ps1s[i] = ps1

        def p1_relu_w2(i):
            ps1 = ps1s.pop(i)
            h1 = p1_sb.tile([128, 512], BF16, tag="h1")
            nc.scalar.activation(h1, ps1, AF.Relu, bias=wf["b1"], scale=1.0)
            ps2 = p1_ps.tile([128, 512], F32, tag="pb")
            nc.tensor.matmul(ps2, wf["w2"], h1, start=True, stop=True)
            ps2s[i] = ps2
            h1s[i] = h1

        def p1_exit(i):
            ps2 = ps2s.pop(i)
            h1s.pop(i)
            if i < MH:
                hm = p1_sb.tile([128, 512], BF16, tag="hm")
                nc.vector.tensor_scalar(
                    out=hm, in0=ps2, scalar1=wf["b2"], scalar2=None, op0=ALU.add,
                )
                ps3 = p1_ps.tile([128, 512], F32, tag="pc")
                nc.tensor.matmul(ps3, wf["wqk"], hm, start=True, stop=True)
                nc.scalar.activation(qTp[:, i, :], ps3, AF.Copy, bias=0.0, scale=1.0)
            else:
                j = i - MH
                nc.vector.tensor_scalar(
                    out=hT[:, ts_blk(j), :].rearrange("p a b -> p (a b)"),
                    in0=ps2, scalar1=wf["b2"], scalar2=None, op0=ALU.add,
                )

        def p1_transp(i):
            if i < MH:
                return
            j = i - MH
            tp = p1_tp.tile([128, 4, 128], BF16, tag="tp")
            for a in range(4):
                nc.tensor.transpose(tp[:, a, :], hT[:, j * 4 + a, :], ident_bf)
            nc.vector.tensor_copy(
                out=hblk[:, ts_blk(j), :].rearrange("p a b -> p (a b)"),
                in_=tp.rearrange("p a b -> p (a b)"),
            )

        for i in range(NTOT + 2):
            if i < NTOT:
                p1_w1(i)
            if 1 <= i <= NTOT:
                p1_relu_w2(i - 1)
            if 2 <= i <= NTOT + 1:
                p1_exit(i - 2)
                p1_transp(i - 2)

        ph1.close()

        # ================= phase 2: masked attention =================
        # software-pipelined: scores lead by 2 blocks, agg/den trail, so the
        # PE queue never blocks on the exp->mask chain.
        agg_ps_pool = ctx.enter_context(tc.tile_pool(name="agg", bufs=1, space="PSUM"))
        ph2 = ExitStack()
        adj_pool = ph2.enter_context(tc.tile_pool(name="adj", bufs=4))
        e_pool = ph2.enter_context(tc.tile_pool(name="e", bufs=6))
        w_pool = ph2.enter_context(tc.tile_pool(name="w", bufs=8))
        pair_pool = ph2.enter_context(tc.tile_pool(name="pair", bufs=2))
        quad_pool = ph2.enter_context(tc.tile_pool(name="quad", bufs=2))
        scp_ps = ph2.enter_context(tc.tile_pool(name="scp", bufs=3, space="PSUM"))

        aggT = agg_ps_pool.tile([128, MH, 512], F32)

        scps = {}
        wbs = {}
        pairs = {}

        adj4 = {}

        def emit_scores(g):
            if g % 4 == 0:
                a4 = adj_pool.tile([128, 4, m_core], BF16, tag="a")
                nc.sync.dma_start(
                    out=a4,
                    in_=adjt[ts(g // 4, 512), :].rearrange("(a p) m -> p a m", p=128),
                )
                adj4[g // 4] = a4
            scp = scp_ps.tile([128, MH, 512], F32, tag="s")
            for j in range(MH):
                nc.tensor.matmul(
                    scp[:, j, :], hT[:, g, :], qTp[:, j, :], start=True, stop=True
                )
            scps[g] = (scp, adj4[g // 4][:, g % 4, :])

        def emit_mask(g):
            scp, adjb = scps.pop(g)
            eb = e_pool.tile([128, MH, 512], BF16, tag="e")
            nc.scalar.activation(
                eb.rearrange("p a b -> p (a b)"),
                scp.rearrange("p a b -> p (a b)"),
                AF.Exp, bias=0.0, scale=1.0,
            )
            wb = w_pool.tile([128, MH, 512], BF16, tag="w")
            nc.vector.tensor_tensor(
                out=wb.rearrange("p a b -> p (a b)"),
                in0=eb.rearrange("p a b -> p (a b)"),
                in1=adjb,
                op=ALU.mult,
            )
            wbs[g] = wb

        def emit_agg(g):
            wb = wbs.pop(g)
            for j in range(MH):
                nc.tensor.matmul(
                    aggT[:, j, :], hblk[:, g, :], wb[:, j, :],
                    start=(g == 0), stop=(g == NB - 1),
                )
            # den: bf16 pair/quad tree on DVE, quad folded into f32 by GpSimd
            if g % 2 == 1:
                wprev = wbs_keep.pop(g - 1)
                pr = pair_pool.tile([128, MH, 512], BF16, tag="p")
                nc.vector.tensor_tensor(
                    out=pr.rearrange("p a b -> p (a b)"),
                    in0=wprev.rearrange("p a b -> p (a b)"),
                    in1=wb.rearrange("p a b -> p (a b)"),
                    op=ALU.add,
                )
                pairs[(g - 1) // 2] = pr
                if g % 4 == 3:
                    pa = pairs.pop(g // 4 * 2)
                    pb = pairs.pop(g // 4 * 2 + 1)
                    qd = quad_pool.tile([128, MH, 512], BF16, tag="q")
                    nc.vector.tensor_tensor(
                        out=qd.rearrange("p a b -> p (a b)"),
                        in0=pa.rearrange("p a b -> p (a b)"),
                        in1=pb.rearrange("p a b -> p (a b)"),
                        op=ALU.add,
                    )
                    eng = nc.vector if g >= NB - 8 else nc.gpsimd
                    if g == 3:
                        eng.tensor_copy(
                            out=vsum_g.rearrange("p a b -> p (a b)"),
                            in_=qd.rearrange("p a b -> p (a b)"),
                        )
                    else:
                        eng.tensor_tensor(
                            out=vsum_g.rearrange("p a b -> p (a b)"),
                            in0=qd.rearrange("p a b -> p (a b)"),
                            in1=vsum_g.rearrange("p a b -> p (a b)"),
                            op=ALU.add,
                        )
            else:
                wbs_keep[g] = wb

        wbs_keep = {}
        LEAD = 3
        for g in range(NB + LEAD):
            if g < NB:
                emit_scores(g)
            if g >= 1 and g - 1 < NB:
                emit_mask(g - 1)
            if g >= LEAD:
                emit_agg(g - LEAD)

        ph2.close()

        # ================= head =================
        head_sb = ctx.enter_context(tc.tile_pool(name="head_sb", bufs=2))
        head_ps = ctx.enter_context(tc.tile_pool(name="head_ps", bufs=1, space="PSUM"))
        den_ps_pool = ctx.enter_context(tc.tile_pool(name="den", bufs=1, space="PSUM"))

        den_ps = den_ps_pool.tile([1, MH, 512], F32)
        for j in range(MH):
            nc.tensor.matmul(
                den_ps[:, j, :], ones_col_r, vsum_g[:, j, :],
                start=True, stop=True,
            )
        # reciprocal on 128 partitions: den [1,1024] -> denT [128,8] -> recip
        rscr = head_sb.tile([1, MH * 512], F32, tag="rs")
        rden = head_sb.tile([1, MH, 512], F32, tag="rd")
        nc.vector.reciprocal_approx_accurate(
            out=rden.rearrange("p a b -> p (a b)"),
            in_=den_ps.rearrange("p a b -> p (a b)"),
            scratch=rscr,
        )
        rden_r = head_sb.tile([1, MH, 512], F32R, tag="rdr")
        nc.vector.tensor_copy(
            out=rden_r.rearrange("p a b -> p (a b)"),
            in_=rden.rearrange("p a b -> p (a b)"),
        )
        for j in range(MH):
            rdb_ps = head_ps.tile([128, 512], F32, tag="rp")
            nc.tensor.matmul(rdb_ps, ones_row_r, rden_r[:, j, :], start=True, stop=True)
            rdb = head_sb.tile([128, 512], BF16, tag="rdb")
            nc.vector.tensor_copy(out=rdb, in_=rdb_ps)
            normT = head_sb.tile([128, 512], BF16, tag="n")
            nc.vector.scalar_tensor_tensor(
                out=normT, in0=aggT[:, j, :], scalar=1.0, in1=rdb,
                op0=ALU.mult, op1=ALU.mult,
            )
            h3_ps = head_ps.tile([128, 512], F32, tag="h3p")
            nc.tensor.matmul(h3_ps, wf["wh1"], normT, start=True, stop=True)
            h3 = head_sb.tile([128, 512], BF16, tag="h3")
            nc.scalar.activation(h3, h3_ps, AF.Relu, bias=wf["bh1"], scale=1.0)
            oT_ps = head_ps.tile([8, 512], F32, tag="otp")
            nc.tensor.matmul(oT_ps, wf["wh2"], h3, start=True, stop=True)
            oT = head_sb.tile([8, 512], F32, tag="oT")
            nc.vector.tensor_scalar(
                out=oT, in0=oT_ps, scalar1=wf["bh2"], scalar2=None, op0=ALU.add,
            )
            o_ps = head_ps.tile([128, 4, A], F32, tag="op")
            for q in range(4):
                nc.tensor.transpose(o_ps[:, q, :], oT[:, ts(q, 128)], ident_f[0:8, 0:8])
            o_sb = head_sb.tile([128, 4, A], F32, tag="ob")
            nc.vector.tensor_copy(out=o_sb, in_=o_ps)
            nc.scalar.dma_start(
                out=outb[ts(j, 512), :].rearrange("(a p) c -> p a c", p=128),
                in_=o_sb,
            )


# ----------------------------------------------------------------------------
# Host entry point: full inputs in, full output out. 8-way row sharding.
# ----------------------------------------------------------------------------
import numpy as np
import ml_dtypes

N_TOTAL = 8192
N_CORES = 8
M_CORE = N_TOTAL // N_CORES

_cached = {}


def _get_nc():
    if "nc" not in _cached:
        _cached["nc"] = build_kernel(n_total=N_TOTAL, m_core=M_CORE)
    return _cached["nc"]


def _bf16(x):
    return np.ascontiguousarray(
        np.asarray(x, dtype=np.float32).astype(ml_dtypes.bfloat16)
    )


def make_in_maps(state_matrix, adj, w1, b1, w2, b2, wq, wk, wh1, bh1, wh2, bh2):
    f32c = lambda x: np.ascontiguousarray(np.asarray(x, dtype=np.float32))
    stT = _bf16(np.asarray(state_matrix, dtype=np.float32).T)
    wqk = np.asarray(wq, dtype=np.float32) @ np.asarray(wk, dtype=np.float32).T
    wqk = _bf16(wqk / np.float32(np.sqrt(E)))
    adjt_full = _bf16(np.asarray(adj).T)
    common = {
        "stT": stT,
        "w1": _bf16(w1), "w2": _bf16(w2), "wqk": wqk,
        "wh1": _bf16(wh1), "wh2": _bf16(wh2),
        "b1": f32c(b1).reshape(E, 1), "b2": f32c(b2).reshape(E, 1),
        "bh1": f32c(bh1).reshape(E, 1), "bh2": f32c(bh2).reshape(A, 1),
    }
    in_maps = []
    for c in range(N_CORES):
        rows = slice(c * M_CORE, (c + 1) * M_CORE)
        in_maps.append(
            dict(
                common,
                stTm=np.ascontiguousarray(stT[:, rows]),
                adjt=np.ascontiguousarray(adjt_full[:, rows]),
            )
        )
    return in_maps


def kernel(state_matrix, adj, w1, b1, w2, b2, wq, wk, wh1, bh1, wh2, bh2):
    from concourse import bass_utils

    in_maps = make_in_maps(
        state_matrix, adj, w1, b1, w2, b2, wq, wk, wh1, bh1, wh2, bh2
    )
    res = bass_utils.run_bass_kernel_spmd(
        _get_nc(), in_maps, core_ids=list(range(N_CORES))
    )
    out = np.concatenate([r["outb"] for r in res.results], axis=0)
    return out.astype(np.float32)


# revision 23
# speedup vs baseline: 1.0066x; 1.0066x over previous
"""CoLightNet Trainium2 Bass kernel (self-contained).

SPMD over 8 cores; core c owns output rows [c*1024, (c+1)*1024).
  inputs : stT   [S,N]    bf16  state transposed (host prep)
           stTm  [S,Mc]   bf16  own-rows slice of stT
           adjt  [N,Mc]   bf16  adjt[n,m] = adj[row m, col n]  (transposed)
           w1,w2,wqk,wh1 [128,128] bf16, wh2 [128,8] bf16
           b1,b2,bh1 [E,1] f32, bh2 [A,1] f32
  output : outb  [Mc,A]   f32

Math (identical to the reference, reformulated):
  hT    = w2^T relu(w1^T stT + b1) + b2                  # [E, N]
  qTp   = Wqk^T hTm            (Wqk = wq wk^T / sqrt(E)) # [E, Mc]
  sT    = hT-block (stationary) x qTp (moving)           # scores^T [N, Mc]
  e     = exp(sT);  w = e * adjt                         # post-exp mask
  den   = sum_n w   (PE matmul / DVE / GpSimd accumulators, merged in PSUM)
  aggT  = sum_blocks h-block (stationary) x w-block      # [E, Mc]
  out   = relu((aggT/den)^T wh1 + bh1) wh2 + bh2
"""

from contextlib import ExitStack

import concourse.bass as bass
import concourse.mybir as mybir
import concourse.tile as tile
from concourse import bacc
from concourse.masks import make_identity

F32 = mybir.dt.float32
F32R = mybir.dt.float32r
BF16 = mybir.dt.bfloat16
AF = mybir.ActivationFunctionType
ALU = mybir.AluOpType

S = 128
E = 128
A = 8


def ts(i, size):
    return slice(i * size, (i + 1) * size)


def ts_blk(j):
    return slice(j * 4, (j + 1) * 4)


# den ownership per n-block, period 8: PE x2, GpSimd x3, DVE x3
DEN_PAT = ["P", "G", "D", "G", "D", "G", "P", "D"]


def build_kernel(n_total=8192, m_core=1024):
    nc = bacc.Bacc("TRN2", debug=False)
    stT = nc.dram_tensor("stT", (S, n_total), BF16, kind="ExternalInput").ap()
    stTm = nc.dram_tensor("stTm", (S, m_core), BF16, kind="ExternalInput").ap()
    adjt = nc.dram_tensor("adjt", (n_total, m_core), BF16, kind="ExternalInput").ap()
    wt = {}
    for name, shape, dt in [
        ("w1", (S, E), BF16), ("w2", (E, E), BF16), ("wqk", (E, E), BF16),
        ("wh1", (E, E), BF16), ("wh2", (E, A), BF16),
        ("b1", (E, 1), F32), ("b2", (E, 1), F32),
        ("bh1", (E, 1), F32), ("bh2", (A, 1), F32),
    ]:
        wt[name] = nc.dram_tensor(name, shape, dt, kind="ExternalInput").ap()
    outb = nc.dram_tensor("outb", (m_core, A), F32, kind="ExternalOutput").ap()

    with tile.TileContext(nc) as tc:
        colight_body(tc, outb, stT, stTm, adjt, wt)
    nc.compile()
    return nc


def colight_body(tc, outb, stT, stTm, adjt, wt):
    nc = tc.nc
    n_total = stT.shape[1]
    m_core = adjt.shape[1]
    NCH = n_total // 512            # 512-wide n chunks (phase 1)
    NB = n_total // 128             # 128-wide n blocks (phase 2)
    MH = m_core // 512              # m halves

    with ExitStack() as ctx:
        singles = ctx.enter_context(tc.tile_pool(name="singles", bufs=1))

        # ---- staged inputs: big DMAs issued first (cheap issue, deep prefetch) ----
        stTm_sb = singles.tile([128, m_core], BF16, tag="stTm")
        nc.sync.dma_start(out=stTm_sb, in_=stTm)
        stT_sb = singles.tile([128, n_total], BF16, tag="stT")
        for q in range(4):
            nc.sync.dma_start(
                out=stT_sb[:, ts(q, n_total // 4)], in_=stT[:, ts(q, n_total // 4)]
            )

        # ---- constant weights ----
        wf = {}
        for name, shape, dt in [
            ("w1", [S, E], BF16), ("w2", [E, E], BF16), ("wqk", [E, E], BF16),
            ("wh1", [E, E], BF16), ("wh2", [E, A], BF16),
            ("b1", [E, 1], F32), ("b2", [E, 1], F32),
            ("bh1", [E, 1], F32), ("bh2", [A, 1], F32),
        ]:
            t = singles.tile(shape, dt, tag=f"w_{name}")
            nc.scalar.dma_start(out=t, in_=wt[name])
            wf[name] = t
        ones_col_f = singles.tile([128, 1], F32)    # vsum-reduce stationary
        nc.vector.memset(ones_col_f, 1.0)
        ones_col_r = singles.tile([128, 1], F32R)
        nc.vector.tensor_copy(out=ones_col_r, in_=ones_col_f)
        ones_col_bf = singles.tile([128, 1], BF16)
        nc.vector.tensor_copy(out=ones_col_bf, in_=ones_col_f)
        ones_row_f = singles.tile([1, 128], F32)    # outer-product stationary
        nc.vector.memset(ones_row_f, 1.0)
        ones_row_r = singles.tile([1, 128], F32R)
        nc.vector.tensor_copy(out=ones_row_r, in_=ones_row_f)
        ident_f = singles.tile([128, 128], F32)
        make_identity(nc, ident_f)
        ident_bf = singles.tile([128, 128], BF16)
        nc.vector.tensor_copy(out=ident_bf, in_=ident_f)

        # ---- persistent activations ----
        hT = singles.tile([128, NB, 128], BF16)     # h^T, [E, n]
        hblk = singles.tile([128, NB, 128], BF16)   # h normal, agg stationary
        qTp = singles.tile([128, MH, 512], BF16)    # Wqk^T hTm
        vsum_g = singles.tile([128, MH, 512], F32R)  # den accumulator (GpSimd folds)

        # ================= phase 1: MLP over all N =================
        ph1 = ExitStack()
        p1_sb = ph1.enter_context(tc.tile_pool(name="p1_sb", bufs=3))
        p1_ps = ph1.enter_context(tc.tile_pool(name="p1_ps", bufs=2, space="PSUM"))
        p1_tp = ph1.enter_context(tc.tile_pool(name="p1_tp", bufs=2, space="PSUM"))

        # one pipelined pass over MH own-row chunks (-> qTp) + NCH full chunks
        # (-> hT, hblk).  stage skew keeps every engine queue unblocked:
        #   step i: w1(i) | relu(i-1) | w2(i-1) | exit(i-1) | transp(i-2)
        NTOT = MH + NCH
        ps1s = {}
        h1s = {}
        ps2s = {}

        def src_dma(i):
            if i < MH:
                return stTm_sb[:, ts(i, 512)]
            return stT_sb[:, ts(i - MH, 512)]

        def p1_w1(i):
            st_in = src_dma(i)
            ps1 = p1_ps.tile([128, 512], F32, tag="pa")
            nc.tensor.matmul(ps1, wf["w1"], st_in, start=True, stop=True)
            ps1s[i] = ps1

        def p1_relu_w2(i):
            ps1 = ps1s.pop(i)
            h1 = p1_sb.tile([128, 512], BF16, tag="h1")
            nc.scalar.activation(h1, ps1, AF.Relu, bias=wf["b1"], scale=1.0)
            ps2 = p1_ps.tile([128, 512], F32, tag="pb")
            nc.tensor.matmul(ps2, wf["w2"], h1, start=True, stop=True)
            ps2s[i] = ps2
            h1s[i] = h1

        def p1_exit(i):
            ps2 = ps2s.pop(i)
            h1s.pop(i)
            if i < MH:
                hm = p1_sb.tile([128, 512], BF16, tag="hm")
                nc.vector.tensor_scalar(
                    out=hm, in0=ps2, scalar1=wf["b2"], scalar2=None, op0=ALU.add,
                )
                ps3 = p1_ps.tile([128, 512], F32, tag="pc")
                nc.tensor.matmul(ps3, wf["wqk"], hm, start=True, stop=True)
                nc.scalar.activation(qTp[:, i, :], ps3, AF.Copy, bias=0.0, scale=1.0)
            else:
                j = i - MH
                nc.vector.tensor_scalar(
                    out=hT[:, ts_blk(j), :].rearrange("p a b -> p (a b)"),
                    in0=ps2, scalar1=wf["b2"], scalar2=None, op0=ALU.add,
                )

        def p1_transp(i):
            if i < MH:
                return
            j = i - MH
            tp = p1_tp.tile([128, 4, 128], BF16, tag="tp")
            for a in range(4):
                nc.tensor.transpose(tp[:, a, :], hT[:, j * 4 + a, :], ident_bf)
            nc.vector.tensor_copy(
                out=hblk[:, ts_blk(j), :].rearrange("p a b -> p (a b)"),
                in_=tp.rearrange("p a b -> p (a b)"),
            )

        for i in range(NTOT + 2):
            if i < NTOT:
                p1_w1(i)
            if 1 <= i <= NTOT:
                p1_relu_w2(i - 1)
            if 2 <= i <= NTOT + 1:
                p1_exit(i - 2)
                p1_transp(i - 2)

        ph1.close()

        # ================= phase 2: masked attention =================
        # software-pipelined: scores lead by 2 blocks, agg/den trail, so the
        # PE queue never blocks on the exp->mask chain.
        agg_ps_pool = ctx.enter_context(tc.tile_pool(name="agg", bufs=1, space="PSUM"))
        den_ps_pool = ctx.enter_context(tc.tile_pool(name="den", bufs=1, space="PSUM"))
        ph2 = ExitStack()
        adj_pool = ph2.enter_context(tc.tile_pool(name="adj", bufs=4))
        e_pool = ph2.enter_context(tc.tile_pool(name="e", bufs=6))
        w_pool = ph2.enter_context(tc.tile_pool(name="w", bufs=8))
        pair_pool = ph2.enter_context(tc.tile_pool(name="pair", bufs=2))
        quad_pool = ph2.enter_context(tc.tile_pool(name="quad", bufs=2))
        scp_ps = ph2.enter_context(tc.tile_pool(name="scp", bufs=2, space="PSUM"))

        aggT = agg_ps_pool.tile([128, MH, 512], F32)
        den_ps = den_ps_pool.tile([1, MH, 512], F32)
        NPE_DEN = NB // 2   # g < NPE_DEN: den via PE matmul; rest: bf16 tree

        scps = {}
        wbs = {}
        pairs = {}

        adj4 = {}

        def emit_scores(g):
            if g % 4 == 0:
                a4 = adj_pool.tile([128, 4, m_core], BF16, tag="a")
                nc.sync.dma_start(
                    out=a4,
                    in_=adjt[ts(g // 4, 512), :].rearrange("(a p) m -> p a m", p=128),
                )
                adj4[g // 4] = a4
            scp = scp_ps.tile([128, MH, 512], F32, tag="s")
            for j in range(MH):
                nc.tensor.matmul(
                    scp[:, j, :], hT[:, g, :], qTp[:, j, :], start=True, stop=True
                )
            scps[g] = (scp, adj4[g // 4][:, g % 4, :])

        def emit_mask(g):
            scp, adjb = scps.pop(g)
            eb = e_pool.tile([128, MH, 512], BF16, tag="e")
            nc.scalar.activation(
                eb.rearrange("p a b -> p (a b)"),
                scp.rearrange("p a b -> p (a b)"),
                AF.Exp, bias=0.0, scale=1.0,
            )
            wb = w_pool.tile([128, MH, 512], BF16, tag="w")
            nc.vector.tensor_tensor(
                out=wb.rearrange("p a b -> p (a b)"),
                in0=eb.rearrange("p a b -> p (a b)"),
                in1=adjb,
                op=ALU.mult,
            )
            wbs[g] = wb

        def emit_agg(g):
            wb = wbs.pop(g)
            for j in range(MH):
                nc.tensor.matmul(
                    aggT[:, j, :], hblk[:, g, :], wb[:, j, :],
                    start=(g == 0), stop=(g == NB - 1),
                )
            # den: first half accumulated on PE into PSUM; second half via
            # bf16 pair/quad tree on DVE with GpSimd folds into vsum_g
            if g < NPE_DEN:
                for j in range(MH):
                    nc.tensor.matmul(
                        den_ps[:, j, :], ones_col_bf, wb[:, j, :],
                        start=(g == 0), stop=False, skip_group_check=True,
                    )
            elif g % 2 == 1:
                wprev = wbs_keep.pop(g - 1)
                pr = pair_pool.tile([128, MH, 512], BF16, tag="p")
                nc.vector.tensor_tensor(
                    out=pr.rearrange("p a b -> p (a b)"),
                    in0=wprev.rearrange("p a b -> p (a b)"),
                    in1=wb.rearrange("p a b -> p (a b)"),
                    op=ALU.add,
                )
                pairs[(g - 1) // 2] = pr
                if g % 4 == 3:
                    pa = pairs.pop(g // 4 * 2)
                    pb = pairs.pop(g // 4 * 2 + 1)
                    qd = quad_pool.tile([128, MH, 512], BF16, tag="q")
                    nc.vector.tensor_tensor(
                        out=qd.rearrange("p a b -> p (a b)"),
                        in0=pa.rearrange("p a b -> p (a b)"),
                        in1=pb.rearrange("p a b -> p (a b)"),
                        op=ALU.add,
                    )
                    eng = nc.vector if g >= NB - 8 else nc.gpsimd
                    if g == NPE_DEN + 3:
                        eng.tensor_copy(
                            out=vsum_g.rearrange("p a b -> p (a b)"),
                            in_=qd.rearrange("p a b -> p (a b)"),
                        )
                    else:
                        eng.tensor_tensor(
                            out=vsum_g.rearrange("p a b -> p (a b)"),
                            in0=qd.rearrange("p a b -> p (a b)"),
                            in1=vsum_g.rearrange("p a b -> p (a b)"),
                            op=ALU.add,
                        )
            else:
                wbs_keep[g] = wb

        wbs_keep = {}
        LEAD = 2
        for g in range(NB + LEAD):
            if g < NB:
                emit_scores(g)
            if g >= 1 and g - 1 < NB:
                emit_mask(g - 1)
            if g >= LEAD:
                emit_agg(g - LEAD)

        ph2.close()

        # ================= head =================
        head_sb = ctx.enter_context(tc.tile_pool(name="head_sb", bufs=2))
        head_ps = ctx.enter_context(tc.tile_pool(name="head_ps", bufs=1, space="PSUM"))

        for j in range(MH):
            nc.tensor.matmul(
                den_ps[:, j, :], ones_col_r, vsum_g[:, j, :],
                start=False, stop=(j == MH - 1), skip_group_check=True,
            )
        # reciprocal on 128 partitions: den [1,1024] -> denT [128,8] -> recip
        rscr = head_sb.tile([1, MH * 512], F32, tag="rs")
        rden = head_sb.tile([1, MH, 512], F32, tag="rd")
        nc.vector.reciprocal_approx_accurate(
            out=rden.rearrange("p a b -> p (a b)"),
            in_=den_ps.rearrange("p a b -> p (a b)"),
            scratch=rscr,
        )
        rden_r = head_sb.tile([1, MH, 512], F32R, tag="rdr")
        nc.vector.tensor_copy(
            out=rden_r.rearrange("p a b -> p (a b)"),
            in_=rden.rearrange("p a b -> p (a b)"),
        )
        for j in range(MH):
            rdb_ps = head_ps.tile([128, 512], F32, tag="rp")
            nc.tensor.matmul(rdb_ps, ones_row_r, rden_r[:, j, :], start=True, stop=True)
            rdb = head_sb.tile([128, 512], BF16, tag="rdb")
            nc.vector.tensor_copy(out=rdb, in_=rdb_ps)
            normT = head_sb.tile([128, 512], BF16, tag="n")
            nc.vector.scalar_tensor_tensor(
                out=normT, in0=aggT[:, j, :], scalar=1.0, in1=rdb,
                op0=ALU.mult, op1=ALU.mult,
            )
            h3_ps = head_ps.tile([128, 512], F32, tag="h3p")
            nc.tensor.matmul(h3_ps, wf["wh1"], normT, start=True, stop=True)
            h3 = head_sb.tile([128, 512], BF16, tag="h3")
            nc.scalar.activation(h3, h3_ps, AF.Relu, bias=wf["bh1"], scale=1.0)
            oT_ps = head_ps.tile([8, 512], F32, tag="otp")
            nc.tensor.matmul(oT_ps, wf["wh2"], h3, start=True, stop=True)
            oT = head_sb.tile([8, 512], F32, tag="oT")
            nc.vector.tensor_scalar(
                out=oT, in0=oT_ps, scalar1=wf["bh2"], scalar2=None, op0=ALU.add,
            )
            o_ps = head_ps.tile([128, 4, A], F32, tag="op")
            for q in range(4):
                nc.tensor.transpose(o_ps[:, q, :], oT[:, ts(q, 128)], ident_f[0:8, 0:8])
            o_sb = head_sb.tile([128, 4, A], F32, tag="ob")
            nc.vector.tensor_copy(out=o_sb, in_=o_ps)
            nc.scalar.dma_start(
                out=outb[ts(j, 512), :].rearrange("(a p) c -> p a c", p=128),
                in_=o_sb,
            )


# ----------------------------------------------------------------------------
# Host entry point: full inputs in, full output out. 8-way row sharding.
# ----------------------------------------------------------------------------
import numpy as np
import ml_dtypes

N_TOTAL = 8192
N_CORES = 8
M_CORE = N_TOTAL // N_CORES

_cached = {}


def _get_nc():
    if "nc" not in _cached:
        _cached["nc"] = build_kernel(n_total=N_TOTAL, m_core=M_CORE)
    return _cached["nc"]


def _bf16(x):
    return np.ascontiguousarray(
        np.asarray(x, dtype=np.float32).astype(ml_dtypes.bfloat16)
    )


def make_in_maps(state_matrix, adj, w1, b1, w2, b2, wq, wk, wh1, bh1, wh2, bh2):
    f32c = lambda x: np.ascontiguousarray(np.asarray(x, dtype=np.float32))
    stT = _bf16(np.asarray(state_matrix, dtype=np.float32).T)
    wqk = np.asarray(wq, dtype=np.float32) @ np.asarray(wk, dtype=np.float32).T
    wqk = _bf16(wqk / np.float32(np.sqrt(E)))
    adjt_full = _bf16(np.asarray(adj).T)
    common = {
        "stT": stT,
        "w1": _bf16(w1), "w2": _bf16(w2), "wqk": wqk,
        "wh1": _bf16(wh1), "wh2": _bf16(wh2),
        "b1": f32c(b1).reshape(E, 1), "b2": f32c(b2).reshape(E, 1),
        "bh1": f32c(bh1).reshape(E, 1), "bh2": f32c(bh2).reshape(A, 1),
    }
    in_maps = []
    for c in range(N_CORES):
        rows = slice(c * M_CORE, (c + 1) * M_CORE)
        in_maps.append(
            dict(
                common,
                stTm=np.ascontiguousarray(stT[:, rows]),
                adjt=np.ascontiguousarray(adjt_full[:, rows]),
            )
        )
    return in_maps


def kernel(state_matrix, adj, w1, b1, w2, b2, wq, wk, wh1, bh1, wh2, bh2):
    from concourse import bass_utils

    in_maps = make_in_maps(
        state_matrix, adj, w1, b1, w2, b2, wq, wk, wh1, bh1, wh2, bh2
    )
    res = bass_utils.run_bass_kernel_spmd(
        _get_nc(), in_maps, core_ids=list(range(N_CORES))
    )
    out = np.concatenate([r["outb"] for r in res.results], axis=0)
    return out.astype(np.float32)
